# revision 1
# baseline (speedup 1.0000x reference)
"""Causal self-attention on 8 TRN2 NeuronCores (Bass/Tile, SPMD).

Problem: B=4, T=2048, C=1024, H=16, D=64, fp32 in/out.

Sharding: core i = (batch b=i//2, parity p=i%2). Each core computes ALL 16
heads for its interleaved quarter of query positions: 256-wide q-chunks
{0,3,4,7} (parity 0) or {1,2,5,6} (parity 1) of batch b. Slot-sorted by
causal prefix length, both parities' slots pad to extents {4,8,12,16}
t'-tiles -> every core runs the IDENTICAL instruction stream (SPMD), with
causality/padding handled by host-supplied mask data. K/V are computed for
the full sequence on both cores of a batch (cheap duplication beats any
collective here). No inter-core communication at all.

Per-core pipeline:
  1. K^T[d,t], Q^T[d,t_own] (d on partitions, heads packed 2/tile) and
     V_aug[t,(h,d|1)] (ones column folded in for softmax sums) via fp32r
     matmuls (1 cyc/row at N>=256; measured rel err 1.5e-4 at K=1024).
  2. Flash-style attention per (head-pair, q-slot): S^T = K @ Q^T with
     2-head row-packed matmuls (K=64 via tile_position), batched exp on
     ScalarE over [128,1024] PSUM spans, causal mask-mul on DVE for the
     last 4 t'-tiles of each slot, PV accumulation O^T = V_aug.T @ P^T
     (M=65: row 64 = softmax denominators l). Normalize with fast DVE
     reciprocal + GpSimd partition-broadcast.
  3. Output projection Y = O^T.T @ Wproj + bias_eff, where
     bias_eff = bproj + bv @ Wproj (V-bias folded in exactly since
     softmax rows sum to 1).

Host: slices/transposes inputs, precomputes masks + effective bias,
reassembles the 8 per-core [1024,1024] outputs into [4,2048,1024].
"""
import os
import numpy as np

import concourse.bacc as bacc
import concourse.mybir as mybir
import concourse.tile as tile
from concourse.bass_utils import run_bass_kernel_spmd

B, T, C, H, D = 4, 2048, 1024, 16, 64
QC = 256                      # q-chunk width
NSLOT = 4                     # q-chunks per core
OWN = [[0, 3, 4, 7], [1, 2, 5, 6]]   # global q-chunk ids per parity, slot order
EXT = [4, 8, 12, 16]          # padded t'-tile (128) extent per slot
F32 = mybir.dt.float32
F32R = mybir.dt.float32r
VA_W = H * (D + 1)            # 1040: V_aug cols = 16 heads x (64 + ones)

_cache = {}


def _build():
    nc = bacc.Bacc("TRN2", target_bir_lowering=False, debug=False,
                   enable_asserts=False, num_devices=8)
    dt_in = {}
    def din(name, shape):
        dt_in[name] = nc.dram_tensor(name, list(shape), F32, kind="ExternalInput").ap()
        return dt_in[name]

    xt_d = din("xt", (C, T))            # x[b].T
    xq_d = din("xq", (C, NSLOT * QC))   # own q columns of xt
    wq_d = din("wq", (C, C))            # pre-scaled by 1/8
    wk_d = din("wk", (C, C))
    wv_d = din("wv", (C, C))
    wp_d = din("wp", (C, C))
    bq_d = din("bq", (8, 128, 1))
    bk_d = din("bk", (8, 128, 1))
    bpeb_d = din("bpeb", (128, C))      # bproj_eff broadcast to 128 partitions
    mk_d = din("masks", (NSLOT, 4, 128, QC))
    y_d = nc.dram_tensor("y", [NSLOT * QC, C], F32, kind="ExternalOutput").ap()
    qt_d = nc.dram_tensor("qt", [C, NSLOT * QC], F32R).ap()
    ot_d = nc.dram_tensor("ot", [C, NSLOT * QC], F32R).ap()

    bypass = mybir.AluOpType.bypass
    mult = mybir.AluOpType.mult
    add = mybir.AluOpType.add
    EXP = mybir.ActivationFunctionType.Exp

    with tile.TileContext(nc) as tc:
        # ---------------- persistent K^T tiles --------------------------
        ktp = tc.alloc_tile_pool(name="ktp", bufs=1)
        KT = [ktp.tile([128, T], F32R, name=f"kt{j}", tag=f"kt{j}") for j in range(8)]

        # ---------------- phase 1a: K^T and Q^T -------------------------
        with tc.tile_pool(name="p1a", bufs=1) as wpool, \
             tc.tile_pool(name="p1ax", bufs=2) as xsp, \
             tc.tile_pool(name="p1ae", bufs=3) as evp, \
             tc.tile_pool(name="p1ap", bufs=1, space="PSUM") as ps1:
            wkc = [wpool.tile([128, C], F32R, name=f"wkc{c}", tag=f"wkc{c}") for c in range(8)]
            wqc = [wpool.tile([128, C], F32R, name=f"wqc{c}", tag=f"wqc{c}") for c in range(8)]
            bks = [wpool.tile([128, 1], F32, name=f"bks{j}", tag=f"bks{j}") for j in range(8)]
            bqs = [wpool.tile([128, 1], F32, name=f"bqs{j}", tag=f"bqs{j}") for j in range(8)]
            for c in range(8):
                nc.sync.dma_start(out=wkc[c][:], in_=wk_d[128*c:128*(c+1), :].bitcast(F32R))
                nc.sync.dma_start(out=wqc[c][:], in_=wq_d[128*c:128*(c+1), :].bitcast(F32R))
                nc.sync.dma_start(out=bks[c][:], in_=bk_d[c])
                nc.sync.dma_start(out=bqs[c][:], in_=bq_d[c])
            # K^T: 4 t-slabs of 512
            for slab in range(4):
                xts = []
                for c in range(8):
                    xt_t = xsp.tile([128, 512], F32R, name=f"xts{c}", tag=f"xts{c}")
                    nc.sync.dma_start(out=xt_t[:], in_=xt_d[128*c:128*(c+1), 512*slab:512*(slab+1)].bitcast(F32R))
                    xts.append(xt_t)
                pks = [ps1.tile([128, 512], F32, name=f"pk{j}", tag=f"pk{j}") for j in range(8)]
                for c in range(8):
                    for j in range(8):
                        nc.tensor.matmul(out=pks[j][:], lhsT=wkc[c][:, 128*j:128*(j+1)],
                                         rhs=xts[c][:], start=(c == 0), stop=(c == 7))
                for j in range(8):
                    nc.vector.tensor_scalar_add(out=KT[j][:, 512*slab:512*(slab+1)],
                                                in0=pks[j][:], scalar1=bks[j][:])
            # Q^T: 2 t-slabs of 512 over own columns
            for slab in range(2):
                xqs = []
                for c in range(8):
                    xq_t = xsp.tile([128, 512], F32R, name=f"xts{c}", tag=f"xts{c}")
                    nc.sync.dma_start(out=xq_t[:], in_=xq_d[128*c:128*(c+1), 512*slab:512*(slab+1)].bitcast(F32R))
                    xqs.append(xq_t)
                pqs = [ps1.tile([128, 512], F32, name=f"pk{j}", tag=f"pk{j}") for j in range(8)]
                for c in range(8):
                    for j in range(8):
                        nc.tensor.matmul(out=pqs[j][:], lhsT=wqc[c][:, 128*j:128*(j+1)],
                                         rhs=xqs[c][:], start=(c == 0), stop=(c == 7))
                for j in range(8):
                    qsb = evp.tile([128, 512], F32R, name="qsb", tag="qsb")
                    nc.vector.tensor_scalar_add(out=qsb[:], in0=pqs[j][:], scalar1=bqs[j][:])
                    nc.sync.dma_start(out=qt_d[128*j:128*(j+1), 512*slab:512*(slab+1)], in_=qsb[:])

        # ---------------- phase 1b: V_aug -------------------------------
        vap = tc.alloc_tile_pool(name="vap", bufs=1)
        VA = [vap.tile([128, VA_W], F32R, name=f"va{g}", tag=f"va{g}") for g in range(16)]
        with tc.tile_pool(name="p1b", bufs=1) as wvp, \
             tc.tile_pool(name="p1bx", bufs=1) as xsp2, \
             tc.tile_pool(name="p1bp", bufs=1, space="PSUM") as ps2:
            wvc = [wvp.tile([128, C], F32R, name=f"wvc{c}", tag=f"wvc{c}") for c in range(8)]
            for c in range(8):
                nc.sync.dma_start(out=wvc[c][:], in_=wv_d[128*c:128*(c+1), :].bitcast(F32R))
            ones16 = wvp.tile([128, H], F32, name="ones16", tag="ones16")
            nc.vector.memset(ones16[:], 1.0)
            ones16_3d = ones16[:].unsqueeze(2)
            for g in range(16):
                dst1 = VA[g][:].rearrange("p (h d) -> p h d", d=D+1)[:, :, D:D+1]
                nc.vector.tensor_copy(out=dst1, in_=ones16_3d)
            for slab in range(4):
                xts2 = []
                for c in range(8):
                    xv_t = xsp2.tile([128, 512], F32R, name=f"xv{c}", tag=f"xv{c}")
                    nc.sync.dma_start(out=xv_t[:], in_=xt_d[128*c:128*(c+1), 512*slab:512*(slab+1)].bitcast(F32R))
                    xts2.append(xv_t)
                pvs = [ps2.tile([128, 512], F32, name=f"pv{u}", tag=f"pv{u}") for u in range(8)]
                for c in range(8):
                    for tt in range(4):
                        for jc in range(2):
                            nc.tensor.matmul(out=pvs[tt*2+jc][:],
                                             lhsT=xts2[c][:, 128*tt:128*(tt+1)],
                                             rhs=wvc[c][:, 512*jc:512*(jc+1)],
                                             start=(c == 0), stop=(c == 7))
                for tt in range(4):
                    g = 4*slab + tt
                    for jc in range(2):
                        dst = VA[g][:, 520*jc:520*(jc+1)].rearrange("p (h d) -> p h d", d=D+1)[:, :, 0:D]
                        src = pvs[tt*2+jc][:].rearrange("p (h d) -> p h d", d=D)
                        nc.vector.tensor_copy(out=dst, in_=src)

        # ---------------- phase 2: attention ----------------------------
        with tc.tile_pool(name="mkp", bufs=1) as mkp, \
             tc.tile_pool(name="qrp", bufs=3) as qrp, \
             tc.tile_pool(name="ptp", bufs=3) as ptp, \
             tc.tile_pool(name="smp", bufs=2) as smp, \
             tc.tile_pool(name="p2p", bufs=1, space="PSUM") as psa:
            MK = []
            for s in range(NSLOT):
                row = []
                for mi in range(4):
                    mt = mkp.tile([128, QC], F32R, name=f"mk{s}{mi}", tag=f"mk{s}{mi}")
                    nc.sync.dma_start(out=mt[:], in_=mk_d[s, mi].bitcast(F32R))
                    row.append(mt)
                MK.append(row)
            for s in range(NSLOT):
                E = EXT[s]
                for j in range(8):
                    qr = qrp.tile([128, QC], F32R, name="qr", tag="qr")
                    nc.sync.dma_start(out=qr[:], in_=qt_d[128*j:128*(j+1), QC*s:QC*(s+1)])
                    oa = psa.tile([65, QC], F32, name="oa", tag="oa", bufs=2)
                    ob = psa.tile([65, QC], F32, name="ob", tag="ob", bufs=2)
                    for g in range(E // 2):
                        ss = psa.tile([128, 4*QC], F32, name="ss", tag="ss", bufs=2)
                        for u in range(2):
                            m = 2*g + u
                            nc.tensor.matmul(out=ss[:, QC*u:QC*(u+1)],
                                             lhsT=KT[j][0:64, 128*m:128*(m+1)],
                                             rhs=qr[0:64, :], tile_position=(0, 0),
                                             start=True, stop=True)
                            nc.tensor.matmul(out=ss[:, 2*QC+QC*u:2*QC+QC*(u+1)],
                                             lhsT=KT[j][64:128, 128*m:128*(m+1)],
                                             rhs=qr[64:128, :], tile_position=(64, 0),
                                             start=True, stop=True)
                        pt = ptp.tile([128, 4*QC], F32R, name="pt", tag="pt")
                        nc.scalar.activation(out=pt[:], in_=ss[:], func=EXP)
                        for u in range(2):
                            m = 2*g + u
                            for half, h in ((0, 2*j), (1, 2*j + 1)):
                                pcol = (2*half + u) * QC
                                psl = pt[:, pcol:pcol+QC]
                                if m >= E - 4:
                                    nc.vector.scalar_tensor_tensor(
                                        out=psl, in0=psl, scalar=0.0, in1=MK[s][m-(E-4)][:],
                                        op0=bypass, op1=mult)
                                nc.tensor.matmul(out=(oa if half == 0 else ob)[:],
                                                 lhsT=VA[m][:, 65*h:65*(h+1)],
                                                 rhs=psl,
                                                 start=(m == 0), stop=(m == E - 1))
                    # normalize: r = 1/l, broadcast, multiply; write O^T
                    for half, (acc, h) in enumerate(((oa, 2*j), (ob, 2*j + 1))):
                        lsb = smp.tile([1, QC], F32, name="lsb", tag=f"lsb{half}")
                        nc.vector.tensor_copy(out=lsb[:], in_=acc[64:65, :])
                        rsb = smp.tile([1, QC], F32, name="rsb", tag=f"rsb{half}")
                        nc.vector.reciprocal_approx_fast(rsb[:], lsb[:])
                        rbb = smp.tile([64, QC], F32, name="rbb", tag=f"rbb{half}")
                        nc.gpsimd.partition_broadcast(rbb[:], rsb[:])
                        osb = smp.tile([64, QC], F32R, name="osb", tag=f"osb{half}")
                        nc.vector.scalar_tensor_tensor(out=osb[:], in0=acc[0:64, :],
                                                       scalar=0.0, in1=rbb[:],
                                                       op0=bypass, op1=mult)
                        nc.sync.dma_start(out=ot_d[64*h:64*(h+1), QC*s:QC*(s+1)], in_=osb[:])

        # ---------------- phase 3: output projection --------------------
        with tc.tile_pool(name="p3w", bufs=1) as wpp, \
             tc.tile_pool(name="p3o", bufs=2) as lop, \
             tc.tile_pool(name="p3y", bufs=3) as yp, \
             tc.tile_pool(name="p3p", bufs=2, space="PSUM") as ps3:
            wpc = [wpp.tile([128, C], F32R, name=f"wpc{c}", tag=f"wpc{c}") for c in range(8)]
            bpeb = wpp.tile([128, C], F32, name="bpeb", tag="bpeb")
            nc.sync.dma_start(out=bpeb[:], in_=bpeb_d[:])
            for c in range(8):
                nc.sync.dma_start(out=wpc[c][:], in_=wp_d[128*c:128*(c+1), :].bitcast(F32R))
            for ti in range(8):
                lots = []
                for c in range(8):
                    lot = lop.tile([128, 128], F32R, name=f"lot{c}", tag=f"lot{c}")
                    nc.sync.dma_start(out=lot[:], in_=ot_d[128*c:128*(c+1), 128*ti:128*(ti+1)])
                    lots.append(lot)
                for jc in range(2):
                    py = ps3.tile([128, 512], F32, name="py", tag="py")
                    for c in range(8):
                        nc.tensor.matmul(out=py[:], lhsT=lots[c][:],
                                         rhs=wpc[c][:, 512*jc:512*(jc+1)],
                                         start=(c == 0), stop=(c == 7))
                    ysb = yp.tile([128, 512], F32, name="ysb", tag="ysb")
                    nc.vector.scalar_tensor_tensor(out=ysb[:], in0=py[:], scalar=0.0,
                                                   in1=bpeb[:, 512*jc:512*(jc+1)],
                                                   op0=bypass, op1=add)
                    nc.sync.dma_start(out=y_d[128*ti:128*(ti+1), 512*jc:512*(jc+1)], in_=ysb[:])
        vap.release()
        ktp.release()

    nc.compile()
    return nc


def _get_nc():
    if "nc" not in _cache:
        _cache["nc"] = _build()
    return _cache["nc"]


def _host_prep(x, Wqkv, bqkv, Wproj, bproj):
    x = np.ascontiguousarray(np.asarray(x, dtype=np.float32))
    Wqkv = np.asarray(Wqkv, dtype=np.float32)
    bqkv = np.asarray(bqkv, dtype=np.float32)
    Wproj = np.ascontiguousarray(np.asarray(Wproj, dtype=np.float32))
    bproj = np.asarray(bproj, dtype=np.float32)

    wq = np.ascontiguousarray(Wqkv[:, :C] * np.float32(0.125))
    wk = np.ascontiguousarray(Wqkv[:, C:2*C])
    wv = np.ascontiguousarray(Wqkv[:, 2*C:])
    bq8 = (bqkv[:C] * np.float32(0.125)).reshape(8, 128, 1).copy()
    bk8 = bqkv[C:2*C].reshape(8, 128, 1).copy()
    bv = bqkv[2*C:]
    bpe = (bproj.astype(np.float64) + bv.astype(np.float64) @ Wproj.astype(np.float64)).astype(np.float32)
    bpeb = np.ascontiguousarray(np.broadcast_to(bpe, (128, C)))

    pidx = np.arange(128)[:, None]
    fidx = np.arange(QC)[None, :]
    masks = []
    for par in range(2):
        mk = np.zeros((NSLOT, 4, 128, QC), dtype=np.float32)
        for s, cchunk in enumerate(OWN[par]):
            for mi in range(4):
                g = EXT[s] - 4 + mi
                mk[s, mi] = ((128*g + pidx) <= (QC*cchunk + fidx)).astype(np.float32)
        masks.append(mk)

    in_maps = []
    for core in range(8):
        b, par = core // 2, core % 2
        xt = np.ascontiguousarray(x[b].T)
        xq = np.ascontiguousarray(
            np.concatenate([xt[:, QC*c:QC*(c+1)] for c in OWN[par]], axis=1))
        in_maps.append(dict(xt=xt, xq=xq, wq=wq, wk=wk, wv=wv, wp=Wproj,
                            bq=bq8, bk=bk8, bpeb=bpeb, masks=masks[par]))
    return in_maps


def kernel(x, Wqkv, bqkv, Wproj, bproj):
    nc = _get_nc()
    in_maps = _host_prep(x, Wqkv, bqkv, Wproj, bproj)
    trace = bool(os.environ.get("BASS_TRACE"))
    res = run_bass_kernel_spmd(nc, in_maps, list(range(8)), trace=trace)
    _cache["last_exec_time_ns"] = res.exec_time_ns
    _cache["last_res"] = res
    out = np.empty((B, T, C), dtype=np.float32)
    for core in range(8):
        b, par = core // 2, core % 2
        y = res.results[core]["y"]
        for s, cchunk in enumerate(OWN[par]):
            out[b, QC*cchunk:QC*(cchunk+1)] = y[QC*s:QC*(s+1)]
    return out



# revision 15
# speedup vs baseline: 2.0744x; 2.0744x over previous
"""Causal self-attention on 8 TRN2 NeuronCores (Bass/Tile, SPMD), v2.

Problem: B=4, T=2048, C=1024, H=16, D=64, fp32 in/out.

Sharding: core i = (batch b=i//2, head-half hh=i%2). Each core computes its
8 heads over ALL T=2048 positions of its batch — no K/V duplication, no
padding (exact causal prefixes), identical instruction stream on every core.
The output projection is computed against the core's own 512 O-channels,
giving a partial y[2048,1024]; the host sums the two partials per batch
(tensor-parallel reduce done host-side — there is no device collective).

All matmul operands are bf16 (fp32 PSUM accumulation): 2x less SBUF/DMA
than fp32r, enables fast weight load, measured end-to-end rel err ~5e-3
(budget 2e-2).

Per-core pipeline (one TileContext, phases interleave via the scheduler):
  V:    V_aug[m][t128, 8*(64|1)] tiles (ones col folded for softmax sums).
  K_j/Q_j: K^T/Q^T[2 heads*64d, T] per head-pair j, bias folded, Q pre-scaled.
  attn(j): per 256-wide q chunk c (prefix E=2c+2 k-tiles): S^T via row-packed
        K=64 matmuls (2 heads concurrent via tile_position), batched exp on
        ScalarE over [128,1024] PSUM, causal masks (2 constant step masks) on
        DVE for the diagonal pair only, PV accumulation into oab[65,512]
        (row 64 = softmax sums l). Normalize with DVE reciprocal + GpSimd
        partition-broadcast; head B writes partitions 64:128 directly
        (DVE partition-base shift, HW-verified).
  proj: y_partial = O^T.T @ Wproj_own + (0.5*bproj + bv_own@Wproj_own).
"""
import os
import sys
import numpy as np
import ml_dtypes

import concourse.bacc as bacc
import concourse.mybir as mybir
import concourse.tile as tile
from concourse.bass_utils import run_bass_kernel_spmd

B, T, C, H, D = 4, 2048, 1024, 16, 64
QC = 256                      # q-chunk width
NC_ = 8                       # q-chunks per core
F32 = mybir.dt.float32
BF16 = mybir.dt.bfloat16
NPBF = ml_dtypes.bfloat16

_cache = {}


def _build():
    nc = bacc.Bacc("TRN2", target_bir_lowering=False, debug=False,
                   enable_asserts=False, num_devices=8)

    def din(name, shape, dt=BF16):
        return nc.dram_tensor(name, list(shape), dt, kind="ExternalInput").ap()

    xt_d = din("xt", (C, T))                 # x[b].T, bf16
    wq_d = din("wq", (C, 512))               # own-head Q cols, pre-scaled 1/8
    wk_d = din("wk", (C, 512))
    wv_d = din("wv", (C, 512))
    wp_d = din("wp", (512, C))               # own-channel Wproj rows
    bq_d = din("bq", (4, 128, 1), F32)
    bk_d = din("bk", (4, 128, 1), F32)
    bpe_d = din("bpe", (128, C), F32)        # 0.5*bproj + bv@Wp_own, bcast
    mk_d = din("masks", (128, 1024))         # fused diagonal masks [m0|m1|m0|m1]
    y_d = nc.dram_tensor("y", [T, C], BF16, kind="ExternalOutput").ap()

    bypass = mybir.AluOpType.bypass
    mult = mybir.AluOpType.mult
    add = mybir.AluOpType.add
    EXP = mybir.ActivationFunctionType.Exp

    with tile.TileContext(nc) as tc:
        kp = tc.alloc_tile_pool(name="kp", bufs=1)
        # persistent SBUF tensors
        xres = [kp.tile([128, T], BF16, name=f"x{c}", tag=f"x{c}") for c in range(8)]
        wvt = [kp.tile([128, 512], BF16, name=f"wv{c}", tag=f"wv{c}") for c in range(8)]
        wkt = [kp.tile([128, 512], BF16, name=f"wk{c}", tag=f"wk{c}") for c in range(8)]
        wqt = [kp.tile([128, 512], BF16, name=f"wq{c}", tag=f"wq{c}") for c in range(8)]
        wpt = [kp.tile([128, C], BF16, name=f"wp{j}", tag=f"wp{j}") for j in range(4)]
        KT = [kp.tile([128, T], BF16, name=f"kt{j}", tag=f"kt{j}") for j in range(4)]
        QT = [kp.tile([128, T], BF16, name=f"qt{j}", tag=f"qt{j}") for j in range(4)]
        OT = [kp.tile([128, T], BF16, name=f"ot{j}", tag=f"ot{j}") for j in range(4)]
        VA = [kp.tile([128, 8 * 65], BF16, name=f"va{m}", tag=f"va{m}") for m in range(16)]
        bqs = [kp.tile([128, 1], F32, name=f"bq{j}", tag=f"bq{j}") for j in range(4)]
        bks = [kp.tile([128, 1], F32, name=f"bk{j}", tag=f"bk{j}") for j in range(4)]
        bpe = kp.tile([128, C], F32, name="bpe", tag="bpe")
        MK = kp.tile([128, 1024], BF16, name="mk", tag="mk")
        ones8 = kp.tile([128, 8], BF16, name="ones8", tag="ones8")

        # ---- input DMAs (emission order = fetch priority) ----
        for c in range(8):
            nc.sync.dma_start(out=wvt[c][:], in_=wv_d[128*c:128*(c+1), :])
        for s in range(4):
            for c in range(8):
                nc.sync.dma_start(out=xres[c][:, 512*s:512*(s+1)],
                                  in_=xt_d[128*c:128*(c+1), 512*s:512*(s+1)])
        nc.sync.dma_start(out=MK[:], in_=mk_d)
        for c in range(8):
            nc.sync.dma_start(out=wkt[c][:], in_=wk_d[128*c:128*(c+1), :])
            nc.sync.dma_start(out=wqt[c][:], in_=wq_d[128*c:128*(c+1), :])
        for j in range(4):
            nc.sync.dma_start(out=bks[j][:], in_=bk_d[j])
            nc.sync.dma_start(out=bqs[j][:], in_=bq_d[j])
        for j in range(4):
            nc.sync.dma_start(out=wpt[j][:], in_=wp_d[128*j:128*(j+1), :])
        nc.sync.dma_start(out=bpe[:], in_=bpe_d[:])

        nc.vector.memset(ones8[:], 1.0)
        for m in range(16):
            dst = VA[m][:].rearrange("p (h e) -> p h e", e=D+1)[:, :, D:D+1]
            nc.vector.tensor_copy(out=dst, in_=ones8[:].unsqueeze(2))

        with tc.tile_pool(name="ps", bufs=1, space="PSUM") as psp, \
             tc.tile_pool(name="ptp", bufs=3) as ptp, \
             tc.tile_pool(name="evp", bufs=2) as evp, \
             tc.tile_pool(name="ybp", bufs=3) as ybp:

            def v_tiles(ms):
                for m in ms:
                    s, tt = m // 4, m % 4
                    pv = psp.tile([128, 512], F32, name="acc", tag="acc", bufs=2)
                    for c in range(8):
                        nc.tensor.matmul(out=pv[:],
                                         lhsT=xres[c][:, 512*s+128*tt:512*s+128*(tt+1)],
                                         rhs=wvt[c][:], start=(c == 0), stop=(c == 7))
                    dst = VA[m][:].rearrange("p (h e) -> p h e", e=D+1)[:, :, 0:D]
                    nc.vector.tensor_copy(out=dst,
                                          in_=pv[:].rearrange("p (h d) -> p h d", d=D))

            def proj_tile(ti):
                for jc in range(2):
                    py = psp.tile([128, 512], F32, name="acc", tag="acc", bufs=2)
                    for j in range(4):
                        nc.tensor.matmul(out=py[:],
                                         lhsT=OT[j][:, 128*ti:128*(ti+1)],
                                         rhs=wpt[j][:, 512*jc:512*(jc+1)],
                                         start=(j == 0), stop=(j == 3))
                    ysb = ybp.tile([128, 512], BF16, name="ysb", tag="ysb")
                    nc.vector.scalar_tensor_tensor(
                        out=ysb[:], in0=py[:], scalar=0.0,
                        in1=bpe[:, 512*jc:512*(jc+1)], op0=bypass, op1=add)
                    nc.sync.dma_start(out=y_d[128*ti:128*(ti+1), 512*jc:512*(jc+1)],
                                      in_=ysb[:])

            # V slab 0 first so attention j=0 can start early; the rest of V
            # is emitted after attn(0) and fills its exp-wait PE slack.
            v_tiles(range(4))

            # ---- per head-pair: K_j, Q_j then attention ----
            for j in range(4):
                for s in range(4):
                    pk = psp.tile([128, 512], F32, name="acc", tag="acc", bufs=2)
                    for c in range(8):
                        nc.tensor.matmul(out=pk[:], lhsT=wkt[c][:, 128*j:128*(j+1)],
                                         rhs=xres[c][:, 512*s:512*(s+1)],
                                         start=(c == 0), stop=(c == 7))
                    nc.vector.tensor_scalar_add(out=KT[j][:, 512*s:512*(s+1)],
                                                in0=pk[:], scalar1=bks[j][:])
                    pq = psp.tile([128, 512], F32, name="acc", tag="acc", bufs=2)
                    for c in range(8):
                        nc.tensor.matmul(out=pq[:], lhsT=wqt[c][:, 128*j:128*(j+1)],
                                         rhs=xres[c][:, 512*s:512*(s+1)],
                                         start=(c == 0), stop=(c == 7))
                    nc.vector.tensor_scalar_add(out=QT[j][:, 512*s:512*(s+1)],
                                                in0=pq[:], scalar1=bqs[j][:])

                for cq in range(NC_):
                    E = 2 * cq + 2
                    if j == 0 and cq in (2, 4, 6):
                        # emit V tiles just before the first chunks that read
                        # them (program order = semantic order); they also
                        # fill attn(0) exp-wait slack on PE
                        v_tiles(range(2*cq, 2*cq + 4))
                    qA = QT[j][0:64, QC*cq:QC*(cq+1)]
                    qB = QT[j][64:128, QC*cq:QC*(cq+1)]
                    oab = psp.tile([65, 512], F32, name="oab", tag="oab", bufs=2)
                    for g in range(E // 2):
                        ss = psp.tile([128, 1024], F32, name="ss", tag="ss", bufs=2)
                        for u in range(2):
                            m = 2*g + u
                            nc.tensor.matmul(out=ss[:, QC*u:QC*(u+1)],
                                             lhsT=KT[j][0:64, 128*m:128*(m+1)],
                                             rhs=qA, tile_position=(0, 0),
                                             start=True, stop=True)
                            nc.tensor.matmul(out=ss[:, 512+QC*u:512+QC*(u+1)],
                                             lhsT=KT[j][64:128, 128*m:128*(m+1)],
                                             rhs=qB, tile_position=(64, 0),
                                             start=True, stop=True)
                        pt = ptp.tile([128, 1024], BF16, name="pt", tag="pt")
                        nc.scalar.activation(out=pt[:], in_=ss[:], func=EXP)
                        if g == E // 2 - 1:   # diagonal pair: m = 2c, 2c+1
                            nc.vector.scalar_tensor_tensor(
                                out=pt[:], in0=pt[:], scalar=0.0,
                                in1=MK[:], op0=bypass, op1=mult)
                        for u in range(2):
                            m = 2*g + u
                            # one has_written group per bank: only the first
                            # matmul starts it; head B's m=0 overwrite relies
                            # on the bank-wide pending-zero from head A's start
                            nc.tensor.matmul(out=oab[:, 0:QC],
                                             lhsT=VA[m][:, 65*(2*j):65*(2*j)+65],
                                             rhs=pt[:, QC*u:QC*(u+1)],
                                             start=(m == 0), stop=(m == E - 1))
                            nc.tensor.matmul(out=oab[:, QC:512],
                                             lhsT=VA[m][:, 65*(2*j+1):65*(2*j+1)+65],
                                             rhs=pt[:, 512+QC*u:512+QC*(u+1)],
                                             start=False, stop=(m == E - 1),
                                             skip_group_check=True)
                    # normalize both heads; head B lands on partitions 64:128
                    lsb = evp.tile([1, 512], F32, name="lsb", tag="lsb")
                    nc.vector.tensor_copy(out=lsb[:], in_=oab[64:65, :])
                    rsb = evp.tile([1, 512], F32, name="rsb", tag="rsb")
                    nc.vector.reciprocal_approx_fast(rsb[:], lsb[:])
                    rbb = evp.tile([64, 512], F32, name="rbb", tag="rbb")
                    nc.gpsimd.partition_broadcast(rbb[:], rsb[:])
                    nc.vector.scalar_tensor_tensor(
                        out=OT[j][0:64, QC*cq:QC*(cq+1)], in0=oab[0:64, 0:QC],
                        scalar=0.0, in1=rbb[:, 0:QC], op0=bypass, op1=mult)
                    nc.vector.scalar_tensor_tensor(
                        out=OT[j][64:128, QC*cq:QC*(cq+1)], in0=oab[0:64, QC:512],
                        scalar=0.0, in1=rbb[:, QC:512], op0=bypass, op1=mult)
                    if j == 3:   # all head-pairs done for this chunk -> project
                        proj_tile(2*cq)
                        proj_tile(2*cq + 1)
        kp.release()

    nc.compile()
    return nc


def _get_nc():
    if "nc" not in _cache:
        _cache["nc"] = _build()
    return _cache["nc"]


def _host_prep(x, Wqkv, bqkv, Wproj, bproj):
    x = np.asarray(x, dtype=np.float32)
    Wqkv = np.asarray(Wqkv, dtype=np.float32)
    bqkv = np.asarray(bqkv, dtype=np.float32)
    Wproj = np.asarray(Wproj, dtype=np.float32)
    bproj = np.asarray(bproj, dtype=np.float32)

    xts = [np.ascontiguousarray(x[b].T).astype(NPBF) for b in range(B)]
    wq_hh, wk_hh, wv_hh, wp_hh, bq_hh, bk_hh, bpe_hh = [], [], [], [], [], [], []
    for hh in range(2):
        sl = slice(512*hh, 512*(hh+1))
        wq_hh.append(np.ascontiguousarray(Wqkv[:, 0:C][:, sl] * np.float32(0.125)).astype(NPBF))
        wk_hh.append(np.ascontiguousarray(Wqkv[:, C:2*C][:, sl]).astype(NPBF))
        wv_hh.append(np.ascontiguousarray(Wqkv[:, 2*C:][:, sl]).astype(NPBF))
        wp_hh.append(np.ascontiguousarray(Wproj[sl, :]).astype(NPBF))
        bq_hh.append((bqkv[0:C][sl] * np.float32(0.125)).reshape(4, 128, 1).copy())
        bk_hh.append(bqkv[C:2*C][sl].reshape(4, 128, 1).copy())
        bv = bqkv[2*C:][sl]
        bpe = (0.5*bproj.astype(np.float64)
               + bv.astype(np.float64) @ Wproj[sl, :].astype(np.float64)).astype(np.float32)
        bpe_hh.append(np.ascontiguousarray(np.broadcast_to(bpe, (128, C))))

    pidx = np.arange(128)[:, None]
    fidx = np.arange(QC)[None, :]
    m0 = (pidx <= fidx)
    m1 = (128 + pidx <= fidx)
    masks = np.ascontiguousarray(
        np.concatenate([m0, m1, m0, m1], axis=1)).astype(NPBF)  # [128,1024]

    in_maps = []
    for core in range(8):
        b, hh = core // 2, core % 2
        in_maps.append(dict(xt=xts[b], wq=wq_hh[hh], wk=wk_hh[hh], wv=wv_hh[hh],
                            wp=wp_hh[hh], bq=bq_hh[hh], bk=bk_hh[hh],
                            bpe=bpe_hh[hh], masks=masks))
    return in_maps


def kernel(x, Wqkv, bqkv, Wproj, bproj):
    nc = _get_nc()
    in_maps = _host_prep(x, Wqkv, bqkv, Wproj, bproj)
    trace = bool(os.environ.get("BASS_TRACE")) and "antenv.axon_hooks" in sys.modules
    res = run_bass_kernel_spmd(nc, in_maps, list(range(8)), trace=trace)
    _cache["last_exec_time_ns"] = res.exec_time_ns
    _cache["last_res"] = res
    out = np.empty((B, T, C), dtype=np.float32)
    for b in range(B):
        out[b] = np.asarray(res.results[2*b]["y"], dtype=np.float32)
        out[b] += np.asarray(res.results[2*b + 1]["y"], dtype=np.float32)
    return out


# revision 16
# speedup vs baseline: 2.0885x; 1.0068x over previous
"""Causal self-attention on 8 TRN2 NeuronCores (Bass/Tile, SPMD), v2.

Problem: B=4, T=2048, C=1024, H=16, D=64, fp32 in/out.

Sharding: core i = (batch b=i//2, head-half hh=i%2). Each core computes its
8 heads over ALL T=2048 positions of its batch — no K/V duplication, no
padding (exact causal prefixes), identical instruction stream on every core.
The output projection is computed against the core's own 512 O-channels,
giving a partial y[2048,1024]; the host sums the two partials per batch
(tensor-parallel reduce done host-side — there is no device collective).

All matmul operands are bf16 (fp32 PSUM accumulation): 2x less SBUF/DMA
than fp32r, enables fast weight load, measured end-to-end rel err ~5e-3
(budget 2e-2).

Per-core pipeline (one TileContext, phases interleave via the scheduler):
  V:    V_aug[m][t128, 8*(64|1)] tiles (ones col folded for softmax sums).
  K_j/Q_j: K^T/Q^T[2 heads*64d, T] per head-pair j, bias folded, Q pre-scaled.
  attn(j): per 256-wide q chunk c (prefix E=2c+2 k-tiles): S^T via row-packed
        K=64 matmuls (2 heads concurrent via tile_position), batched exp on
        ScalarE over [128,1024] PSUM, causal masks (2 constant step masks) on
        DVE for the diagonal pair only, PV accumulation into oab[65,512]
        (row 64 = softmax sums l). Normalize with DVE reciprocal + GpSimd
        partition-broadcast; head B writes partitions 64:128 directly
        (DVE partition-base shift, HW-verified).
  proj: y_partial = O^T.T @ Wproj_own + (0.5*bproj + bv_own@Wproj_own).
"""
import os
import sys
import numpy as np
import ml_dtypes

import concourse.bacc as bacc
import concourse.mybir as mybir
import concourse.tile as tile
from concourse.bass_utils import run_bass_kernel_spmd

B, T, C, H, D = 4, 2048, 1024, 16, 64
QC = 256                      # q-chunk width
NC_ = 8                       # q-chunks per core
F32 = mybir.dt.float32
BF16 = mybir.dt.bfloat16
NPBF = ml_dtypes.bfloat16

_cache = {}


def _build():
    nc = bacc.Bacc("TRN2", target_bir_lowering=False, debug=False,
                   enable_asserts=False, num_devices=8)

    def din(name, shape, dt=BF16):
        return nc.dram_tensor(name, list(shape), dt, kind="ExternalInput").ap()

    xt_d = din("xt", (C, T))                 # x[b].T, bf16
    wq_d = din("wq", (C, 512))               # own-head Q cols, pre-scaled 1/8
    wk_d = din("wk", (C, 512))
    wv_d = din("wv", (C, 512))
    wp_d = din("wp", (512, C))               # own-channel Wproj rows
    bq_d = din("bq", (4, 128, 1), F32)
    bk_d = din("bk", (4, 128, 1), F32)
    bpe_d = din("bpe", (128, C), F32)        # 0.5*bproj + bv@Wp_own, bcast
    mk_d = din("masks", (128, 1024))         # fused diagonal masks [m0|m1|m0|m1]
    y_d = nc.dram_tensor("y", [T, C], BF16, kind="ExternalOutput").ap()

    bypass = mybir.AluOpType.bypass
    mult = mybir.AluOpType.mult
    add = mybir.AluOpType.add
    EXP = mybir.ActivationFunctionType.Exp

    with tile.TileContext(nc) as tc:
        kp = tc.alloc_tile_pool(name="kp", bufs=1)
        # persistent SBUF tensors
        xres = [kp.tile([128, T], BF16, name=f"x{c}", tag=f"x{c}") for c in range(8)]
        wvt = [kp.tile([128, 512], BF16, name=f"wv{c}", tag=f"wv{c}") for c in range(8)]
        wkt = [kp.tile([128, 512], BF16, name=f"wk{c}", tag=f"wk{c}") for c in range(8)]
        wqt = [kp.tile([128, 512], BF16, name=f"wq{c}", tag=f"wq{c}") for c in range(8)]
        wpt = [kp.tile([128, C], BF16, name=f"wp{j}", tag=f"wp{j}") for j in range(4)]
        KT = [kp.tile([128, T], BF16, name=f"kt{j}", tag=f"kt{j}") for j in range(4)]
        QT = [kp.tile([128, T], BF16, name=f"qt{j}", tag=f"qt{j}") for j in range(4)]
        OT = [kp.tile([128, T], BF16, name=f"ot{j}", tag=f"ot{j}") for j in range(4)]
        VA = [kp.tile([128, 8 * 65], BF16, name=f"va{m}", tag=f"va{m}") for m in range(16)]
        bqs = [kp.tile([128, 1], F32, name=f"bq{j}", tag=f"bq{j}") for j in range(4)]
        bks = [kp.tile([128, 1], F32, name=f"bk{j}", tag=f"bk{j}") for j in range(4)]
        bpe = kp.tile([128, C], F32, name="bpe", tag="bpe")
        MK = kp.tile([128, 1024], BF16, name="mk", tag="mk")
        ones8 = kp.tile([128, 8], BF16, name="ones8", tag="ones8")

        # ---- input DMAs: x whole-tile on Sync, weights on Scalar ----
        # (two issuing engines in parallel; big transfers avoid the
        # serialized descriptor + in-order-queue wait cascade)
        for c in range(8):
            nc.sync.dma_start(out=xres[c][:], in_=xt_d[128*c:128*(c+1), :])
        for c in range(8):
            nc.scalar.dma_start(out=wvt[c][:], in_=wv_d[128*c:128*(c+1), :])
        for c in range(8):
            nc.scalar.dma_start(out=wkt[c][:], in_=wk_d[128*c:128*(c+1), :])
            nc.scalar.dma_start(out=wqt[c][:], in_=wq_d[128*c:128*(c+1), :])
        for j in range(4):
            nc.scalar.dma_start(out=bks[j][:], in_=bk_d[j])
            nc.scalar.dma_start(out=bqs[j][:], in_=bq_d[j])
        nc.scalar.dma_start(out=MK[:], in_=mk_d)
        for j in range(4):
            nc.scalar.dma_start(out=wpt[j][:], in_=wp_d[128*j:128*(j+1), :])
        nc.scalar.dma_start(out=bpe[:], in_=bpe_d[:])

        nc.vector.memset(ones8[:], 1.0)
        for m in range(16):
            dst = VA[m][:].rearrange("p (h e) -> p h e", e=D+1)[:, :, D:D+1]
            nc.vector.tensor_copy(out=dst, in_=ones8[:].unsqueeze(2))

        with tc.tile_pool(name="ps", bufs=1, space="PSUM") as psp, \
             tc.tile_pool(name="ptp", bufs=3) as ptp, \
             tc.tile_pool(name="evp", bufs=2) as evp, \
             tc.tile_pool(name="ybp", bufs=3) as ybp:

            def v_tiles(ms):
                for m in ms:
                    s, tt = m // 4, m % 4
                    pv = psp.tile([128, 512], F32, name="acc", tag="acc", bufs=2)
                    for c in range(8):
                        nc.tensor.matmul(out=pv[:],
                                         lhsT=xres[c][:, 512*s+128*tt:512*s+128*(tt+1)],
                                         rhs=wvt[c][:], start=(c == 0), stop=(c == 7))
                    dst = VA[m][:].rearrange("p (h e) -> p h e", e=D+1)[:, :, 0:D]
                    nc.vector.tensor_copy(out=dst,
                                          in_=pv[:].rearrange("p (h d) -> p h d", d=D))

            def proj_tile(ti):
                for jc in range(2):
                    py = psp.tile([128, 512], F32, name="acc", tag="acc", bufs=2)
                    for j in range(4):
                        nc.tensor.matmul(out=py[:],
                                         lhsT=OT[j][:, 128*ti:128*(ti+1)],
                                         rhs=wpt[j][:, 512*jc:512*(jc+1)],
                                         start=(j == 0), stop=(j == 3))
                    ysb = ybp.tile([128, 512], BF16, name="ysb", tag="ysb")
                    nc.vector.scalar_tensor_tensor(
                        out=ysb[:], in0=py[:], scalar=0.0,
                        in1=bpe[:, 512*jc:512*(jc+1)], op0=bypass, op1=add)
                    nc.sync.dma_start(out=y_d[128*ti:128*(ti+1), 512*jc:512*(jc+1)],
                                      in_=ysb[:])

            # V slab 0 first so attention j=0 can start early; the rest of V
            # is emitted after attn(0) and fills its exp-wait PE slack.
            v_tiles(range(4))

            # ---- per head-pair: K_j, Q_j then attention ----
            for j in range(4):
                for s in range(4):
                    pk = psp.tile([128, 512], F32, name="acc", tag="acc", bufs=2)
                    for c in range(8):
                        nc.tensor.matmul(out=pk[:], lhsT=wkt[c][:, 128*j:128*(j+1)],
                                         rhs=xres[c][:, 512*s:512*(s+1)],
                                         start=(c == 0), stop=(c == 7))
                    nc.vector.tensor_scalar_add(out=KT[j][:, 512*s:512*(s+1)],
                                                in0=pk[:], scalar1=bks[j][:])
                    pq = psp.tile([128, 512], F32, name="acc", tag="acc", bufs=2)
                    for c in range(8):
                        nc.tensor.matmul(out=pq[:], lhsT=wqt[c][:, 128*j:128*(j+1)],
                                         rhs=xres[c][:, 512*s:512*(s+1)],
                                         start=(c == 0), stop=(c == 7))
                    nc.vector.tensor_scalar_add(out=QT[j][:, 512*s:512*(s+1)],
                                                in0=pq[:], scalar1=bqs[j][:])

                for cq in range(NC_):
                    E = 2 * cq + 2
                    if j == 0 and cq in (2, 4, 6):
                        # emit V tiles just before the first chunks that read
                        # them (program order = semantic order); they also
                        # fill attn(0) exp-wait slack on PE
                        v_tiles(range(2*cq, 2*cq + 4))
                    qA = QT[j][0:64, QC*cq:QC*(cq+1)]
                    qB = QT[j][64:128, QC*cq:QC*(cq+1)]
                    oab = psp.tile([65, 512], F32, name="oab", tag="oab", bufs=2)
                    for g in range(E // 2):
                        ss = psp.tile([128, 1024], F32, name="ss", tag="ss", bufs=2)
                        for u in range(2):
                            m = 2*g + u
                            nc.tensor.matmul(out=ss[:, QC*u:QC*(u+1)],
                                             lhsT=KT[j][0:64, 128*m:128*(m+1)],
                                             rhs=qA, tile_position=(0, 0),
                                             start=True, stop=True)
                            nc.tensor.matmul(out=ss[:, 512+QC*u:512+QC*(u+1)],
                                             lhsT=KT[j][64:128, 128*m:128*(m+1)],
                                             rhs=qB, tile_position=(64, 0),
                                             start=True, stop=True)
                        pt = ptp.tile([128, 1024], BF16, name="pt", tag="pt")
                        nc.scalar.activation(out=pt[:], in_=ss[:], func=EXP)
                        if g == E // 2 - 1:   # diagonal pair: m = 2c, 2c+1
                            nc.vector.scalar_tensor_tensor(
                                out=pt[:], in0=pt[:], scalar=0.0,
                                in1=MK[:], op0=bypass, op1=mult)
                        for u in range(2):
                            m = 2*g + u
                            # one has_written group per bank: only the first
                            # matmul starts it; head B's m=0 overwrite relies
                            # on the bank-wide pending-zero from head A's start
                            nc.tensor.matmul(out=oab[:, 0:QC],
                                             lhsT=VA[m][:, 65*(2*j):65*(2*j)+65],
                                             rhs=pt[:, QC*u:QC*(u+1)],
                                             start=(m == 0), stop=(m == E - 1))
                            nc.tensor.matmul(out=oab[:, QC:512],
                                             lhsT=VA[m][:, 65*(2*j+1):65*(2*j+1)+65],
                                             rhs=pt[:, 512+QC*u:512+QC*(u+1)],
                                             start=False, stop=(m == E - 1),
                                             skip_group_check=True)
                    # normalize both heads; head B lands on partitions 64:128
                    lsb = evp.tile([1, 512], F32, name="lsb", tag="lsb")
                    nc.vector.tensor_copy(out=lsb[:], in_=oab[64:65, :])
                    rsb = evp.tile([1, 512], F32, name="rsb", tag="rsb")
                    nc.vector.reciprocal_approx_fast(rsb[:], lsb[:])
                    rbb = evp.tile([64, 512], F32, name="rbb", tag="rbb")
                    nc.gpsimd.partition_broadcast(rbb[:], rsb[:])
                    nc.vector.scalar_tensor_tensor(
                        out=OT[j][0:64, QC*cq:QC*(cq+1)], in0=oab[0:64, 0:QC],
                        scalar=0.0, in1=rbb[:, 0:QC], op0=bypass, op1=mult)
                    nc.vector.scalar_tensor_tensor(
                        out=OT[j][64:128, QC*cq:QC*(cq+1)], in0=oab[0:64, QC:512],
                        scalar=0.0, in1=rbb[:, QC:512], op0=bypass, op1=mult)
                    if j == 3:   # all head-pairs done for this chunk -> project
                        proj_tile(2*cq)
                        proj_tile(2*cq + 1)
        kp.release()

    nc.compile()
    return nc


def _get_nc():
    if "nc" not in _cache:
        _cache["nc"] = _build()
    return _cache["nc"]


def _host_prep(x, Wqkv, bqkv, Wproj, bproj):
    x = np.asarray(x, dtype=np.float32)
    Wqkv = np.asarray(Wqkv, dtype=np.float32)
    bqkv = np.asarray(bqkv, dtype=np.float32)
    Wproj = np.asarray(Wproj, dtype=np.float32)
    bproj = np.asarray(bproj, dtype=np.float32)

    xts = [np.ascontiguousarray(x[b].T).astype(NPBF) for b in range(B)]
    wq_hh, wk_hh, wv_hh, wp_hh, bq_hh, bk_hh, bpe_hh = [], [], [], [], [], [], []
    for hh in range(2):
        sl = slice(512*hh, 512*(hh+1))
        wq_hh.append(np.ascontiguousarray(Wqkv[:, 0:C][:, sl] * np.float32(0.125)).astype(NPBF))
        wk_hh.append(np.ascontiguousarray(Wqkv[:, C:2*C][:, sl]).astype(NPBF))
        wv_hh.append(np.ascontiguousarray(Wqkv[:, 2*C:][:, sl]).astype(NPBF))
        wp_hh.append(np.ascontiguousarray(Wproj[sl, :]).astype(NPBF))
        bq_hh.append((bqkv[0:C][sl] * np.float32(0.125)).reshape(4, 128, 1).copy())
        bk_hh.append(bqkv[C:2*C][sl].reshape(4, 128, 1).copy())
        bv = bqkv[2*C:][sl]
        bpe = (0.5*bproj.astype(np.float64)
               + bv.astype(np.float64) @ Wproj[sl, :].astype(np.float64)).astype(np.float32)
        bpe_hh.append(np.ascontiguousarray(np.broadcast_to(bpe, (128, C))))

    pidx = np.arange(128)[:, None]
    fidx = np.arange(QC)[None, :]
    m0 = (pidx <= fidx)
    m1 = (128 + pidx <= fidx)
    masks = np.ascontiguousarray(
        np.concatenate([m0, m1, m0, m1], axis=1)).astype(NPBF)  # [128,1024]

    in_maps = []
    for core in range(8):
        b, hh = core // 2, core % 2
        in_maps.append(dict(xt=xts[b], wq=wq_hh[hh], wk=wk_hh[hh], wv=wv_hh[hh],
                            wp=wp_hh[hh], bq=bq_hh[hh], bk=bk_hh[hh],
                            bpe=bpe_hh[hh], masks=masks))
    return in_maps


def kernel(x, Wqkv, bqkv, Wproj, bproj):
    nc = _get_nc()
    in_maps = _host_prep(x, Wqkv, bqkv, Wproj, bproj)
    trace = bool(os.environ.get("BASS_TRACE")) and "antenv.axon_hooks" in sys.modules
    res = run_bass_kernel_spmd(nc, in_maps, list(range(8)), trace=trace)
    _cache["last_exec_time_ns"] = res.exec_time_ns
    _cache["last_res"] = res
    out = np.empty((B, T, C), dtype=np.float32)
    for b in range(B):
        out[b] = np.asarray(res.results[2*b]["y"], dtype=np.float32)
        out[b] += np.asarray(res.results[2*b + 1]["y"], dtype=np.float32)
    return out


# revision 22
# speedup vs baseline: 2.1700x; 1.0390x over previous
"""Causal self-attention on 8 TRN2 NeuronCores (Bass/Tile, SPMD), v2.

Problem: B=4, T=2048, C=1024, H=16, D=64, fp32 in/out.

Sharding: core i = (batch b=i//2, head-half hh=i%2). Each core computes its
8 heads over ALL T=2048 positions of its batch — no K/V duplication, no
padding (exact causal prefixes), identical instruction stream on every core.
The output projection is computed against the core's own 512 O-channels,
giving a partial y[2048,1024]; the host sums the two partials per batch
(tensor-parallel reduce done host-side — there is no device collective).

All matmul operands are bf16 (fp32 PSUM accumulation): 2x less SBUF/DMA
than fp32r, enables fast weight load, measured end-to-end rel err ~5e-3
(budget 2e-2).

Per-core pipeline (one TileContext, phases interleave via the scheduler):
  V:    V_aug[m][t128, 8*(64|1)] tiles (ones col folded for softmax sums).
  K_j/Q_j: K^T/Q^T[2 heads*64d, T] per head-pair j, bias folded, Q pre-scaled.
  attn(j): per 256-wide q chunk c (prefix E=2c+2 k-tiles): S^T via row-packed
        K=64 matmuls (2 heads concurrent via tile_position), batched exp on
        ScalarE over [128,1024] PSUM, causal masks (2 constant step masks) on
        DVE for the diagonal pair only, PV accumulation into oab[65,512]
        (row 64 = softmax sums l). Normalize with DVE reciprocal + GpSimd
        partition-broadcast; head B writes partitions 64:128 directly
        (DVE partition-base shift, HW-verified).
  proj: y_partial = O^T.T @ Wproj_own + (0.5*bproj + bv_own@Wproj_own).
"""
import os
import sys
import numpy as np
import ml_dtypes

import concourse.bacc as bacc
import concourse.mybir as mybir
import concourse.tile as tile
from concourse.bass_utils import run_bass_kernel_spmd

B, T, C, H, D = 4, 2048, 1024, 16, 64
QC = 256                      # q-chunk width
NC_ = 8                       # q-chunks per core
F32 = mybir.dt.float32
BF16 = mybir.dt.bfloat16
NPBF = ml_dtypes.bfloat16

_cache = {}


def _build():
    nc = bacc.Bacc("TRN2", target_bir_lowering=False, debug=False,
                   enable_asserts=False, num_devices=8)

    def din(name, shape, dt=BF16):
        return nc.dram_tensor(name, list(shape), dt, kind="ExternalInput").ap()

    xt_d = din("xt", (C, T))                 # x[b].T, bf16
    # packed weight/const blobs (f32 pieces bitcast to 2 bf16 cols each):
    # A: wv[8x512]                                           -> 4096 cols
    # B: wk[8x512] | wq[8x512] | bk[4x2] | bq[4x2]           -> 8208 cols
    # C: wp[4x1024] | bpe[2048] | masks[1024]                -> 7168 cols
    U16 = mybir.dt.uint16
    ba_d = din("blob_a", (128, 4096), U16)
    bb_d = din("blob_b", (128, 8208), U16)
    bc_d = din("blob_c", (128, 7168), U16)
    y_d = nc.dram_tensor("y", [T, C], BF16, kind="ExternalOutput").ap()

    bypass = mybir.AluOpType.bypass
    mult = mybir.AluOpType.mult
    add = mybir.AluOpType.add
    EXP = mybir.ActivationFunctionType.Exp

    with tile.TileContext(nc) as tc:
        kp = tc.alloc_tile_pool(name="kp", bufs=1)
        # persistent SBUF tensors
        xres = [kp.tile([128, T], BF16, name=f"x{c}", tag=f"x{c}") for c in range(8)]
        blobA = kp.tile([128, 4096], mybir.dt.uint16, name="blobA", tag="blobA")
        blobB = kp.tile([128, 8208], mybir.dt.uint16, name="blobB", tag="blobB")
        blobC = kp.tile([128, 7168], mybir.dt.uint16, name="blobC", tag="blobC")
        KT = [kp.tile([128, T], BF16, name=f"kt{j}", tag=f"kt{j}") for j in range(4)]
        QT = [kp.tile([128, T], BF16, name=f"qt{j}", tag=f"qt{j}") for j in range(4)]
        OT = [kp.tile([128, T], BF16, name=f"ot{j}", tag=f"ot{j}") for j in range(4)]
        VA = [kp.tile([128, 8 * 65], BF16, name=f"va{m}", tag=f"va{m}") for m in range(16)]
        ones8 = kp.tile([128, 8], BF16, name="ones8", tag="ones8")

        wvt = [blobA[:, 512*c:512*(c+1)].bitcast(BF16) for c in range(8)]
        wkt = [blobB[:, 512*c:512*(c+1)].bitcast(BF16) for c in range(8)]
        wqt = [blobB[:, 4096+512*c:4096+512*(c+1)].bitcast(BF16) for c in range(8)]
        bks = [blobB[:, 8192+2*j:8192+2*(j+1)].bitcast(F32) for j in range(4)]
        bqs = [blobB[:, 8200+2*j:8200+2*(j+1)].bitcast(F32) for j in range(4)]
        wpt = [blobC[:, 1024*j:1024*(j+1)].bitcast(BF16) for j in range(4)]
        bpe = blobC[:, 4096:6144].bitcast(F32)
        MK = blobC[:, 6144:7168].bitcast(BF16)

        # ---- input DMAs: all on Sync; 11 big descriptors total ----
        nc.sync.dma_start(out=blobA[:], in_=ba_d)
        for c in range(8):
            nc.sync.dma_start(out=xres[c][:], in_=xt_d[128*c:128*(c+1), :])
        nc.sync.dma_start(out=blobB[:], in_=bb_d)
        nc.sync.dma_start(out=blobC[:], in_=bc_d)

        nc.vector.memset(ones8[:], 1.0)
        for m in range(16):
            dst = VA[m][:].rearrange("p (h e) -> p h e", e=D+1)[:, :, D:D+1]
            nc.vector.tensor_copy(out=dst, in_=ones8[:].unsqueeze(2))

        with tc.tile_pool(name="ps", bufs=1, space="PSUM") as psp, \
             tc.tile_pool(name="ptp", bufs=3) as ptp, \
             tc.tile_pool(name="evp", bufs=2) as evp, \
             tc.tile_pool(name="ybp", bufs=3) as ybp:

            def v_tiles(ms):
                for m in ms:
                    s, tt = m // 4, m % 4
                    pv = psp.tile([128, 512], F32, name="acc", tag="acc", bufs=2)
                    for c in range(8):
                        nc.tensor.matmul(out=pv[:],
                                         lhsT=xres[c][:, 512*s+128*tt:512*s+128*(tt+1)],
                                         rhs=wvt[c], start=(c == 0), stop=(c == 7))
                    dst = VA[m][:].rearrange("p (h e) -> p h e", e=D+1)[:, :, 0:D]
                    nc.vector.tensor_copy(out=dst,
                                          in_=pv[:].rearrange("p (h d) -> p h d", d=D))

            def proj_tile(ti):
                for jc in range(2):
                    py = psp.tile([128, 512], F32, name="acc", tag="acc", bufs=2)
                    for j in range(4):
                        nc.tensor.matmul(out=py[:],
                                         lhsT=OT[j][:, 128*ti:128*(ti+1)],
                                         rhs=wpt[j][:, 512*jc:512*(jc+1)],
                                         start=(j == 0), stop=(j == 3))
                    ysb = ybp.tile([128, 512], BF16, name="ysb", tag="ysb")
                    nc.vector.scalar_tensor_tensor(
                        out=ysb[:], in0=py[:], scalar=0.0,
                        in1=bpe[:, 512*jc:512*(jc+1)], op0=bypass, op1=add)
                    nc.sync.dma_start(out=y_d[128*ti:128*(ti+1), 512*jc:512*(jc+1)],
                                      in_=ysb[:])

            # V slab 0 first so attention j=0 can start early; the rest of V
            # is emitted after attn(0) and fills its exp-wait PE slack.
            v_tiles(range(4))

            # ---- per head-pair: K_j, Q_j then attention ----
            for j in range(4):
                for s in range(4):
                    pk = psp.tile([128, 512], F32, name="acc", tag="acc", bufs=2)
                    for c in range(8):
                        nc.tensor.matmul(out=pk[:], lhsT=wkt[c][:, 128*j:128*(j+1)],
                                         rhs=xres[c][:, 512*s:512*(s+1)],
                                         start=(c == 0), stop=(c == 7))
                    nc.vector.tensor_scalar_add(out=KT[j][:, 512*s:512*(s+1)],
                                                in0=pk[:], scalar1=bks[j])
                    pq = psp.tile([128, 512], F32, name="acc", tag="acc", bufs=2)
                    for c in range(8):
                        nc.tensor.matmul(out=pq[:], lhsT=wqt[c][:, 128*j:128*(j+1)],
                                         rhs=xres[c][:, 512*s:512*(s+1)],
                                         start=(c == 0), stop=(c == 7))
                    nc.vector.tensor_scalar_add(out=QT[j][:, 512*s:512*(s+1)],
                                                in0=pq[:], scalar1=bqs[j])

                for cq in range(NC_):
                    E = 2 * cq + 2
                    if j == 0 and cq in (2, 4, 6):
                        # emit V tiles just before the first chunks that read
                        # them (program order = semantic order); they also
                        # fill attn(0) exp-wait slack on PE
                        v_tiles(range(2*cq, 2*cq + 4))
                    qA = QT[j][0:64, QC*cq:QC*(cq+1)]
                    qB = QT[j][64:128, QC*cq:QC*(cq+1)]
                    oab = psp.tile([65, 512], F32, name="oab", tag="oab", bufs=2)
                    for g in range(E // 2):
                        ss = psp.tile([128, 1024], F32, name="ss", tag="ss", bufs=2)
                        for u in range(2):
                            m = 2*g + u
                            nc.tensor.matmul(out=ss[:, QC*u:QC*(u+1)],
                                             lhsT=KT[j][0:64, 128*m:128*(m+1)],
                                             rhs=qA, tile_position=(0, 0),
                                             start=True, stop=True)
                            nc.tensor.matmul(out=ss[:, 512+QC*u:512+QC*(u+1)],
                                             lhsT=KT[j][64:128, 128*m:128*(m+1)],
                                             rhs=qB, tile_position=(64, 0),
                                             start=True, stop=True)
                        pt = ptp.tile([128, 1024], BF16, name="pt", tag="pt")
                        nc.scalar.activation(out=pt[:], in_=ss[:], func=EXP)
                        if g == E // 2 - 1:   # diagonal pair: m = 2c, 2c+1
                            nc.vector.scalar_tensor_tensor(
                                out=pt[:], in0=pt[:], scalar=0.0,
                                in1=MK, op0=bypass, op1=mult)
                        for u in range(2):
                            m = 2*g + u
                            # one has_written group per bank: only the first
                            # matmul starts it; head B's m=0 overwrite relies
                            # on the bank-wide pending-zero from head A's start
                            nc.tensor.matmul(out=oab[:, 0:QC],
                                             lhsT=VA[m][:, 65*(2*j):65*(2*j)+65],
                                             rhs=pt[:, QC*u:QC*(u+1)],
                                             start=(m == 0), stop=(m == E - 1))
                            nc.tensor.matmul(out=oab[:, QC:512],
                                             lhsT=VA[m][:, 65*(2*j+1):65*(2*j+1)+65],
                                             rhs=pt[:, 512+QC*u:512+QC*(u+1)],
                                             start=False, stop=(m == E - 1),
                                             skip_group_check=True)
                    # normalize both heads; head B lands on partitions 64:128
                    lsb = evp.tile([1, 512], F32, name="lsb", tag="lsb")
                    nc.vector.tensor_copy(out=lsb[:], in_=oab[64:65, :])
                    rsb = evp.tile([1, 512], F32, name="rsb", tag="rsb")
                    nc.vector.reciprocal_approx_fast(rsb[:], lsb[:])
                    rbb = evp.tile([64, 512], F32, name="rbb", tag="rbb")
                    nc.gpsimd.partition_broadcast(rbb[:], rsb[:])
                    nc.vector.scalar_tensor_tensor(
                        out=OT[j][0:64, QC*cq:QC*(cq+1)], in0=oab[0:64, 0:QC],
                        scalar=0.0, in1=rbb[:, 0:QC], op0=bypass, op1=mult)
                    nc.vector.scalar_tensor_tensor(
                        out=OT[j][64:128, QC*cq:QC*(cq+1)], in0=oab[0:64, QC:512],
                        scalar=0.0, in1=rbb[:, QC:512], op0=bypass, op1=mult)
                    if j == 3:   # all head-pairs done for this chunk -> project
                        proj_tile(2*cq)
                        proj_tile(2*cq + 1)
        kp.release()

    nc.compile()
    return nc


def _get_nc():
    if "nc" not in _cache:
        _cache["nc"] = _build()
    return _cache["nc"]


def _host_prep(x, Wqkv, bqkv, Wproj, bproj):
    x = np.asarray(x, dtype=np.float32)
    Wqkv = np.asarray(Wqkv, dtype=np.float32)
    bqkv = np.asarray(bqkv, dtype=np.float32)
    Wproj = np.asarray(Wproj, dtype=np.float32)
    bproj = np.asarray(bproj, dtype=np.float32)

    def f32_as_bf16(a):
        # reinterpret f32 [128,N] as its raw bits: [128,2N] bf16 columns
        return np.ascontiguousarray(a).view(NPBF)

    def chunks(w):  # [1024,512]->[128, 4096]: stack the 8 row-chunks
        return np.concatenate([w[128*c:128*(c+1), :] for c in range(8)], axis=1)

    pidx = np.arange(128)[:, None]
    fidx = np.arange(QC)[None, :]
    m0 = (pidx <= fidx)
    m1 = (128 + pidx <= fidx)
    masks = np.concatenate([m0, m1, m0, m1], axis=1).astype(NPBF)  # [128,1024]

    xts = [np.ascontiguousarray(x[b].T).astype(NPBF) for b in range(B)]
    ba_hh, bb_hh, bc_hh = [], [], []
    for hh in range(2):
        sl = slice(512*hh, 512*(hh+1))
        wq = chunks((Wqkv[:, 0:C][:, sl] * np.float32(0.125)).astype(NPBF))
        wk = chunks(Wqkv[:, C:2*C][:, sl].astype(NPBF))
        wv = chunks(Wqkv[:, 2*C:][:, sl].astype(NPBF))
        wp = np.concatenate(
            [Wproj[sl, :][128*j:128*(j+1), :].astype(NPBF) for j in range(4)], axis=1)
        bq = f32_as_bf16(bqkv[0:C][sl].reshape(4, 128).T * np.float32(0.125))  # [128,8]
        bk = f32_as_bf16(np.ascontiguousarray(bqkv[C:2*C][sl].reshape(4, 128).T))
        bv = bqkv[2*C:][sl]
        bpe = (0.5*bproj.astype(np.float64)
               + bv.astype(np.float64) @ Wproj[sl, :].astype(np.float64)).astype(np.float32)
        bpe_b = f32_as_bf16(np.broadcast_to(bpe, (128, C)).copy())  # [128,2048]
        ba_hh.append(np.ascontiguousarray(wv).view(np.uint16))
        bb_hh.append(np.ascontiguousarray(
            np.concatenate([wk, wq, bk, bq], axis=1)).view(np.uint16))
        bc_hh.append(np.ascontiguousarray(
            np.concatenate([wp, bpe_b, masks], axis=1)).view(np.uint16))

    in_maps = []
    for core in range(8):
        b, hh = core // 2, core % 2
        in_maps.append(dict(xt=xts[b], blob_a=ba_hh[hh], blob_b=bb_hh[hh],
                            blob_c=bc_hh[hh]))
    return in_maps


def kernel(x, Wqkv, bqkv, Wproj, bproj):
    nc = _get_nc()
    in_maps = _host_prep(x, Wqkv, bqkv, Wproj, bproj)
    trace = bool(os.environ.get("BASS_TRACE")) and "antenv.axon_hooks" in sys.modules
    res = run_bass_kernel_spmd(nc, in_maps, list(range(8)), trace=trace)
    _cache["last_exec_time_ns"] = res.exec_time_ns
    _cache["last_res"] = res
    out = np.empty((B, T, C), dtype=np.float32)
    for b in range(B):
        out[b] = np.asarray(res.results[2*b]["y"], dtype=np.float32)
        out[b] += np.asarray(res.results[2*b + 1]["y"], dtype=np.float32)
    return out
